# revision 3
# baseline (speedup 1.0000x reference)
"""MQA attention kernel for Trainium2 (8 NeuronCores, Bass/Tile).

Problem: Q [2,16,2048,64], K/V [2,1,2048,64] fp32, out = softmax(QK^T/8) V.

Sharding: 32 (batch, head) pairs over 8 cores -> 4 heads per core; each core
gets one batch's K/V (replicated across the 4 cores of that batch).

Per-core algorithm (S^T orientation so softmax reduction lands on the free dim
and PV needs no transposition of P):
  - K^T, Q^T built on-chip via PE transposes (d=64 on partitions, zero-padded
    to 128 so all matmuls contract over the full partition dim).
  - S^T[j, q] = (K Q^T) computed in fp32r matmuls (full-rate 4-byte dtype),
    PSUM bank per 128-row j-chunk.
  - exp(s/8) fused with PSUM->SBUF evacuation on the scalar engine (no max
    subtraction: scores/8 ~ N(0,1), exp never overflows fp32).
  - PV uses V augmented with a ones column: one matmul chain yields both
    O^T = V'^T P^T and the softmax denominators (row 64).
  - O'^T transposed back with PE, normalized with DVE reciprocal+mul, DMA out.

The q-rows are processed in an interleaved order (partition p holds rows
16p+c) so every DMA moves contiguous 4KB runs; the same rearrange on the
output store undoes the permutation.
"""

import numpy as np

import concourse.bass as bass
import concourse.mybir as mybir
import concourse.tile as tile
from concourse import bacc
from concourse.bass_utils import run_bass_kernel_spmd
from concourse.masks import make_identity

B, H, S, D = 2, 16, 2048, 64
N_CORES = 8
HPC = (B * H) // N_CORES  # heads per core = 4
P = 128
NJ = S // P               # 16 key chunks of 128
QB = 512                  # queries per block (= max fp32 matmul free dim)
NQB = S // QB             # 4 q-blocks per head
SCALE = 1.0 / float(D) ** 0.5
F32 = mybir.dt.float32
F32R = mybir.dt.float32r  # 4-byte matmul dtype, full streaming rate at N>=256
EXP_GRP = 2               # j-chunks (PSUM banks) per exp ACTIVATE group

_CACHED = {}


def _build_module():
    nc = bacc.Bacc(None)
    q = nc.dram_tensor("q", [HPC, S, D], F32, kind="ExternalInput")
    k = nc.dram_tensor("k", [S, D], F32, kind="ExternalInput")
    v = nc.dram_tensor("v", [S, D], F32, kind="ExternalInput")
    o = nc.dram_tensor("o", [HPC, S, D], F32, kind="ExternalOutput")

    with tile.TileContext(nc) as tc:
        with (
            tc.tile_pool(name="const", bufs=1) as cpool,
            tc.tile_pool(name="nat", bufs=2) as npool,
            tc.tile_pool(name="work", bufs=2) as wpool,
            tc.tile_pool(name="ps", bufs=2, space="PSUM") as pspool,
            tc.tile_pool(name="ps1", bufs=1, space="PSUM") as ps1pool,
        ):
            identity = cpool.tile([P, P], F32)
            make_identity(nc, identity)

            # ---- K^T [128, 2048]: rows 0-63 = K^T, rows 64-127 zero pad ----
            kT = cpool.tile([P, S], F32R)
            nc.vector.memset(kT[64:P, :].bitcast(mybir.dt.uint32), 0)
            k_nat = npool.tile([P, NJ, D], F32, tag="nat")
            nc.sync.dma_start(k_nat[:], k.rearrange("(p c) d -> p c d", p=P))
            for g in range(NJ // 4):
                pst = ps1pool.tile([64, 4, P], F32, tag="tr", bufs=2, name=f"pst_k{g}")
                for t in range(4):
                    nc.tensor.transpose(pst[:, t, :], k_nat[:, 4 * g + t, :], identity)
                nc.vector.tensor_copy(kT[0:64, 512 * g : 512 * (g + 1)], pst[:])

            # ---- V' [128, 16, 65]: V plus a ones column (softmax denom) ----
            vp = cpool.tile([P, NJ, D + 1], F32R)
            nc.vector.memset(vp[:, :, D].bitcast(mybir.dt.uint32), 0x3F800000)
            v_nat = npool.tile([P, NJ, D], F32, tag="nat", name="v_nat")
            nc.sync.dma_start(v_nat[:], v.rearrange("(p c) d -> p c d", p=P))
            nc.vector.tensor_copy(vp[:, :, 0:D], v_nat[:])

            # ---- Q^T double buffer, zero-padded once ----
            qT_tiles = []
            for i in range(2):
                qTt = cpool.tile([P, S], F32R, name=f"qT{i}")
                nc.vector.memset(qTt[64:P, :].bitcast(mybir.dt.uint32), 0)
                qT_tiles.append(qTt)

            for h in range(HPC):
                q_nat = npool.tile([P, NJ, D], F32, tag="nat", name=f"q_nat{h}")
                nc.sync.dma_start(q_nat[:], q[h].rearrange("(p c) d -> p c d", p=P))
                qT = qT_tiles[h % 2]
                for g in range(NJ // 4):
                    pst = ps1pool.tile(
                        [64, 4, P], F32, tag="tr", bufs=2, name=f"pst_q{h}_{g}"
                    )
                    for t in range(4):
                        nc.tensor.transpose(
                            pst[:, t, :], q_nat[:, 4 * g + t, :], identity
                        )
                    nc.vector.tensor_copy(qT[0:64, 512 * g : 512 * (g + 1)], pst[:])

                for qb in range(NQB):
                    qs = qT[:, QB * qb : QB * (qb + 1)]
                    # exp(S^T/8): j-chunk scores into PSUM, scalar engine
                    # evacuates each EXP_GRP-bank group with a fused exp.
                    pT = wpool.tile([P, NJ * QB], F32R, tag="pT", name=f"pT{h}_{qb}")
                    for g in range(NJ // EXP_GRP):
                        sg = pspool.tile(
                            [P, EXP_GRP, QB], F32, tag="sg", name=f"sg{h}_{qb}_{g}"
                        )
                        for i in range(EXP_GRP):
                            j = EXP_GRP * g + i
                            nc.tensor.matmul(
                                sg[:, i, :],
                                lhsT=kT[:, P * j : P * (j + 1)],
                                rhs=qs,
                                start=True,
                                stop=True,
                            )
                        nc.scalar.activation(
                            pT[:, EXP_GRP * QB * g : EXP_GRP * QB * (g + 1)],
                            sg[:],
                            mybir.ActivationFunctionType.Exp,
                            scale=SCALE,
                        )
                    # O'^T [65, 512] = V'^T P^T accumulated over j-chunks
                    pv = ps1pool.tile([D + 1, QB], F32, tag="pv", name=f"pv{h}_{qb}")
                    for c in range(NJ):
                        nc.tensor.matmul(
                            pv[:],
                            lhsT=vp[:, c, :],
                            rhs=pT[:, QB * c : QB * (c + 1)],
                            start=(c == 0),
                            stop=(c == NJ - 1),
                        )
                    oev = wpool.tile([D + 1, QB], F32, tag="oev", name=f"oev{h}_{qb}")
                    nc.vector.tensor_copy(oev[:], pv[:])
                    # transpose back to [q, d], normalize rows by the denom
                    otr = ps1pool.tile(
                        [P, 4, D + 1], F32, tag="tr", bufs=2, name=f"otr{h}_{qb}"
                    )
                    rcp = wpool.tile([P, 4], F32, tag="rcp", name=f"rcp{h}_{qb}")
                    oout = wpool.tile([P, 4, D], F32, tag="oout", name=f"oout{h}_{qb}")
                    for t in range(4):
                        nc.tensor.transpose(
                            otr[:, t, :],
                            oev[:, P * t : P * (t + 1)],
                            identity[0 : D + 1, 0 : D + 1],
                        )
                        nc.vector.reciprocal(rcp[:, t : t + 1], otr[:, t, D : D + 1])
                        nc.vector.tensor_scalar(
                            oout[:, t, :],
                            otr[:, t, 0:D],
                            rcp[:, t : t + 1],
                            None,
                            mybir.AluOpType.mult,
                        )
                    nc.sync.dma_start(
                        o[h].rearrange("(p c) d -> p c d", p=P)[
                            :, 4 * qb : 4 * (qb + 1), :
                        ],
                        oout[:],
                    )
    nc.compile()
    return nc


def _get_module():
    if "nc" not in _CACHED:
        _CACHED["nc"] = _build_module()
    return _CACHED["nc"]


def make_in_maps(Q, K, V):
    """Shard full inputs into per-core input maps (core c -> batch c//4,
    heads 4*(c%4)..4*(c%4)+4)."""
    Q = np.asarray(Q, dtype=np.float32)
    K = np.asarray(K, dtype=np.float32)
    V = np.asarray(V, dtype=np.float32)
    in_maps = []
    for c in range(N_CORES):
        b = c // (N_CORES // B)
        h0 = HPC * (c % (N_CORES // B))
        in_maps.append(
            {
                "q": np.ascontiguousarray(Q[b, h0 : h0 + HPC]),
                "k": np.ascontiguousarray(K[b, 0]),
                "v": np.ascontiguousarray(V[b, 0]),
            }
        )
    return in_maps


def assemble_output(results):
    out = np.empty((B, H, S, D), dtype=np.float32)
    for c in range(N_CORES):
        b = c // (N_CORES // B)
        h0 = HPC * (c % (N_CORES // B))
        out[b, h0 : h0 + HPC] = results[c]["o"]
    return out


def kernel(Q, K, V):
    nc = _get_module()
    res = run_bass_kernel_spmd(nc, make_in_maps(Q, K, V), core_ids=list(range(N_CORES)))
    return assemble_output(res.results)
